# revision 17
# baseline (speedup 1.0000x reference)
"""Bass/Trainium2 kernel for nn_DiagonalTraining (per-anti-diagonal Linear).

Math: for each anti-diagonal i of x[B,S,S] (entries x[b,r,i-r], r<=i),
apply Linear_i (weights W[i,:i+1,:i+1], bias b[i,:i+1]) to the gathered
vector and scatter back reversed. Equivalent to:
    D[b,i,j] = x[b,j,i-j] (j<=i else 0)
    out[b,i,k] = sum_j W[i,k,j] * D[b,i,j] + b[i,k]
    new_x[b,r,c] = out[b,r+c,c] if r+c < S else x[b,r,c]

Device does the einsum (memory-bound: streams the valid triangle of W);
gather/scatter/bias are tiny O(S^2) host ops.

Sharding: interleaved over diagonals — core c owns i = c, c+8, ..., c+504
(slot m holds diagonal 8m+c, padded to length L=8(m+1)). All cores run one
identical SPMD program; padding rows/cols of W and D are zero by
construction so results are exact.

v5: W and D are fp8 e4m3 (W pre-scaled by 64 to stay in e4m3's normal
range; descaled on host). Everything the matmuls read lives in ONE dram
image laid out in consumption order and fetched with 7 large full-height
DMAs on two HW queues (long per-partition descriptor runs keep all 16
DMA engines fed; the first transfer carries D plus the primer groups so
the PE starts the moment it lands). Each group's partial last
contraction chunk (rem = 32/64/96 rows) is stacked with a partner
group's partial into a shared full-height column band and consumed via
matmul tile_position row offsets, removing most zero-row padding.
Groups of 4 slots share a PSUM bank via tile_position col-packing;
PSUM->SBUF copies (vector, casting to bf16) feed 3 batched
consumption-prefix output DMAs.
"""

import sys

sys.path.insert(0, "/opt/trn_rl_repo")

import numpy as np

B = 8
S = 512
NCORES = 8
M = 64  # diagonal slots per core
G = 16  # groups of 4 slots sharing a PSUM bank
LBAR = [8 * (m + 1) for m in range(M)]  # padded diagonal length per slot
NQG = [g // 4 + 1 for g in range(G)]  # contraction chunks per group
GW = [128 * g + 80 for g in range(G)]  # per-chunk cols (sum of 4 slot L's)
REM = [32 * (g + 1) - 128 * (NQG[g] - 1) for g in range(G)]  # last-chunk rows
LG = [32 * (g + 1) for g in range(G)]  # group output width
WSCALE = 64.0
MODE = "fp8"  # informational; kernel always runs the fp8 scheme

# Consumption order: primer first, then largest-first so the PE tracks the
# arrival stream; ends with one mid-size group so the work remaining after
# the last transfer lands is small (the overhead-heavy tiny groups overlap
# the final transfer instead).
ORDER = [3, 2, 15, 14, 13, 12, 11, 10, 9, 7, 6, 5, 4, 1, 0, 8]

# Banding: groups whose last chunk is partial (rem < 128) donate it to a
# shared full-height column band: top member at rows [0, rem), bottom
# member at rows [128-rem_b, 128). BANDS[name] = [top_g, bottom_g].
BANDS = {"A": [14, 12], "B": [13, 9], "C": [10, 8], "D": [6, 4]}
BAND_OF = {}
PSTART = {}
for _n, (_t, _b) in BANDS.items():
    BAND_OF[_t] = _n
    BAND_OF[_b] = _n
    PSTART[_t] = 0
    PSTART[_b] = 128 - REM[_b]
# Inline chunks per group (banded groups keep nq-1 full chunks inline).
NQI = [NQG[g] - (1 if g in BAND_OF else 0) for g in range(G)]

# Image layout: dt image, then per-consumption-order segments; each band
# sits right before its top member (arrives with/before both consumers).
DTCOLS = 32 * sum(NQG)  # 1280; dt col layout: per group, (q, t, b)
SEGS = [("dt", None), ("g", 3), ("g", 2), ("g", 15), ("band", "A"), ("g", 14),
        ("band", "B"), ("g", 13), ("g", 12), ("g", 11), ("band", "C"),
        ("g", 10), ("g", 9), ("g", 7), ("band", "D"), ("g", 6),
        ("g", 5), ("g", 4), ("g", 1), ("g", 0), ("g", 8)]
WOFF = {}   # group -> inline image base col
BOFF = {}   # band name -> base col
DC2 = {}    # group -> dt base col
_off = 0
for _k, _v in SEGS:
    if _k == "dt":
        _off += DTCOLS
    elif _k == "g":
        WOFF[_v] = _off
        _off += NQI[_v] * GW[_v]
    else:
        BOFF[_v] = _off
        _off += GW[BANDS[_v][0]]
WTOT = _off  # 48576
_off = 0
for _g in ORDER:
    DC2[_g] = _off
    _off += NQG[_g] * 4 * B
assert _off == DTCOLS

# Fetch transfers: consumption-contiguous column ranges (end segment index
# into SEGS), ALL on one HW queue so arrival order is exactly consumption
# order at the full per-queue rate (cross-queue interleaving would starve
# the consumption-critical transfer in favor of later data).
#   T0: dt+g3+g2 | T1: g15+A | T2: g14+B | T3: g13+g12 | T4: g11+C
#   T5: g10+g9   | T6: g7..g0 smalls | T7: g8 inline
TSEG = [3, 5, 7, 9, 11, 13, 20, 21]

# Output: out dram columns in consumption order; 3 prefix-batched writes.
OCONS = {}
_off = 0
for _g in ORDER:
    OCONS[_g] = _off
    _off += LG[_g]
OTOT = _off  # 4352
OBATCH = [ORDER[0:4], ORDER[4:8], ORDER[8:12], ORDER[12:16]]

_compiled = {}


def _seg_cols():
    """(start, end) image col range for each entry of SEGS."""
    res = []
    off = 0
    for k, v in SEGS:
        w = DTCOLS if k == "dt" else (
            NQI[v] * GW[v] if k == "g" else GW[BANDS[v][0]]
        )
        res.append((off, off + w))
        off += w
    return res


def build_program():
    """Build the SPMD Bass program (same instructions on all 8 cores)."""
    import concourse.mybir as mybir
    import concourse.tile as tile
    from concourse import bacc

    f8 = mybir.dt.float8e4
    f32 = mybir.dt.float32
    bf16 = mybir.dt.bfloat16

    nc = bacc.Bacc("TRN2")
    wimg = nc.dram_tensor("wimg", [128, WTOT], f8, kind="ExternalInput")
    out = nc.dram_tensor("out", [128, OTOT], bf16, kind="ExternalOutput")

    segc = _seg_cols()

    with tile.TileContext(nc) as tc:
        with (
            tc.tile_pool(name="wpool", bufs=1) as wpool,
            tc.tile_pool(name="opool", bufs=1) as opool,
            tc.tile_pool(name="psum", bufs=8, space="PSUM") as psum_pool,
        ):
            wtile = wpool.tile([128, WTOT], f8)
            s0 = 0
            for s1 in TSEG:
                a, b_ = segc[s0][0], segc[s1 - 1][1]
                nc.sync.dma_start(wtile[0:128, a:b_], wimg[0:128, a:b_])
                s0 = s1

            batch_of = {}
            for bi, batch in enumerate(OBATCH):
                for g in batch:
                    batch_of[g] = bi
            btiles = {}
            bspan = [sum(LG[g] for g in batch) for batch in OBATCH]
            blo = [OCONS[batch[0]] for batch in OBATCH]
            bdone = [0] * len(OBATCH)
            out_engines = [nc.gpsimd, nc.scalar, nc.gpsimd, nc.scalar]

            for g in ORDER:
                nq = NQG[g]
                ps = psum_pool.tile([128, 512], f32, tag="ps")
                # q-major emission: adjacent matmuls hit different PE
                # col-groups, so they overlap even under FIFO dispatch.
                for q in range(nq):
                    for t in range(4):
                        m = 4 * g + t
                        L = LBAR[m]
                        toff = sum(LBAR[4 * g : 4 * g + t])
                        banded = g in BAND_OF and q == nq - 1
                        if banded:
                            p0, rem = PSTART[g], REM[g]
                            base = BOFF[BAND_OF[g]] + toff
                            rhs = wtile[p0 : p0 + rem, base : base + L]
                            lhsT = wtile[
                                p0 : p0 + rem,
                                DC2[g] + (q * 4 + t) * B : DC2[g]
                                + (q * 4 + t + 1) * B,
                            ]
                            pos = (p0, 32 * t)
                        else:
                            base = WOFF[g] + q * GW[g] + toff
                            rhs = wtile[0:128, base : base + L]
                            lhsT = wtile[
                                0:128,
                                DC2[g] + (q * 4 + t) * B : DC2[g]
                                + (q * 4 + t + 1) * B,
                            ]
                            pos = (0, 32 * t)
                        nc.tensor.matmul(
                            ps[32 * t : 32 * t + B, 0:L],
                            lhsT=lhsT,
                            rhs=rhs,
                            start=(q == 0),
                            stop=(q == nq - 1),
                            tile_position=pos,
                        )
                # Cast to bf16 into this group's slice of its batch tile.
                bi = batch_of[g]
                if bi not in btiles:
                    btiles[bi] = opool.tile(
                        [128, bspan[bi]], bf16, tag=f"ob{bi}", name=f"ob{bi}"
                    )
                bt = btiles[bi]
                off = OCONS[g] - blo[bi]
                nc.vector.tensor_copy(
                    bt[0:128, off : off + LG[g]], ps[0:128, 0 : LG[g]]
                )
                bdone[bi] += 1
                if bdone[bi] == len(OBATCH[bi]):
                    out_engines[bi].dma_start(
                        out[:, blo[bi] : blo[bi] + bspan[bi]],
                        bt[0:128, 0 : bspan[bi]],
                    )

    nc.compile()
    return nc


def _get_program():
    if "fp8" not in _compiled:
        _compiled["fp8"] = build_program()
    return _compiled["fp8"]


def _prep_inputs(x, W):
    """Host-side shard prep: gather diagonals of x, pack W SBUF images."""
    import ml_dtypes

    f8 = np.dtype(ml_dtypes.float8_e4m3)
    i_idx = np.arange(S)[:, None]
    r_idx = np.arange(S)[None, :]
    cols = (i_idx - r_idx) % S
    valid = (r_idx <= i_idx)[None]
    D = np.where(valid, x[:, r_idx, cols], np.float32(0.0))  # [B, S(i), S(j)]
    Dq = D.astype(f8)
    Wq = (W * np.float32(WSCALE)).astype(f8)

    in_maps = []
    for c in range(NCORES):
        Wc = Wq[c::8]  # [M, S(k), S(j)]
        Dc = Dq[:, c::8, :]  # [B, M, S(j)]
        WIMG = np.zeros((128, WTOT), dtype=f8)
        for g in range(G):
            nq, nqi = NQG[g], NQI[g]
            # dt: DT[j, DC2[g] + (q*4 + t)*B + b] = Dc[b, 4g+t, 128q + j]
            sl = Dc[:, 4 * g : 4 * g + 4, 0 : 128 * nq]  # [B, 4, 128nq]
            arr = (
                sl.transpose(2, 1, 0)
                .reshape(nq, 128, 4, B)
                .transpose(1, 0, 2, 3)
                .reshape(128, nq * 4 * B)
            )
            if g in BAND_OF:
                # shift the last chunk's D rows to the band partition range
                last = arr[:, (nq - 1) * 32 : nq * 32].copy()
                arr[:, (nq - 1) * 32 :] = 0
                p0, rem = PSTART[g], REM[g]
                arr[p0 : p0 + rem, (nq - 1) * 32 :] = last[0:rem]
            WIMG[:, DC2[g] : DC2[g] + nq * 32] = arr
            # W inline chunks
            col = WOFF[g]
            for t in range(4):
                m = 4 * g + t
                L = LBAR[m]
                blk = Wc[m, 0:L, 0 : 128 * nqi]  # [k=L, j]
                img = (
                    blk.T.reshape(nqi, 128, L)
                    .transpose(1, 0, 2)
                    .reshape(128, nqi * L)
                )
                for q in range(nqi):
                    WIMG[:, col + q * GW[g] + sum(LBAR[4 * g : m]) :
                         col + q * GW[g] + sum(LBAR[4 * g : m]) + L] = img[
                        :, q * L : (q + 1) * L
                    ]
            # banded last chunk into the band region
            if g in BAND_OF:
                p0, rem = PSTART[g], REM[g]
                q = nq - 1
                base = BOFF[BAND_OF[g]]
                for t in range(4):
                    m = 4 * g + t
                    L = LBAR[m]
                    blk = Wc[m, 0:L, 128 * q : 128 * q + rem]  # [L, rem]
                    WIMG[
                        p0 : p0 + rem,
                        base + sum(LBAR[4 * g : m]) : base
                        + sum(LBAR[4 * g : m]) + L,
                    ] = blk.T
        in_maps.append({"wimg": WIMG})
    return in_maps


def _postprocess(x, bvec, results):
    """Assemble per-core outputs, descale, add bias, scatter back."""
    out_full = np.empty((B, S, S), dtype=np.float32)
    for c in range(NCORES):
        o = np.asarray(results[c]["out"]).astype(np.float32)  # [128, OTOT]
        for g in range(G):
            blk = o[:, OCONS[g] : OCONS[g] + LG[g]].reshape(4, 32, LG[g])[:, 0:B]
            for t in range(4):
                m = 4 * g + t
                out_full[:, 8 * m + c, 0 : LBAR[m]] = blk[t, :, 0 : LBAR[m]]
    out_full *= np.float32(1.0 / WSCALE)
    out_full += bvec[None]
    rr = np.arange(S)[:, None]
    cc = np.arange(S)[None, :]
    diag = rr + cc
    new_x = np.where(
        (diag < S)[None], out_full[:, np.minimum(diag, S - 1), cc], x
    ).astype(np.float32)
    return new_x


def kernel_run(x, W, b, mode=None, trace=False):
    from concourse.bass_utils import run_bass_kernel_spmd

    nc = _get_program()
    in_maps = _prep_inputs(x, W)
    res = run_bass_kernel_spmd(nc, in_maps, list(range(NCORES)), trace=trace)
    return _postprocess(x, b, res.results), res


def kernel(x, W, b):
    out, _ = kernel_run(np.asarray(x), np.asarray(W), np.asarray(b))
    return out
